# revision 44
# baseline (speedup 1.0000x reference)
"""DeepGESNCell kernel v3 for 8 TRN2 NeuronCores.

h <- tanh(wiu + L @ (h @ W_hh^T)) iterated 10x, two layers, out = [h1|h2].

v3 vs the 794us v2 baseline (2-pass packed double-bf16):
  1. Pass 1 of the big matmul runs in float32r (TF32-like, 1 cycle/row for
     512-col moving -- probe-measured 228ns/matmul, same as bf16): the
     stationary packs [h_hi | h_lo] f32r limbs (h exact to wire precision),
     the moving is L_hi = tf32-rounded L (10 explicit mantissa bits,
     pre-rounded on host so the HW f32r rounding is a no-op).
     32 matmuls/iter instead of 64.
  2. Pass 2 adds the L residual: L_lo = L - L_hi quantized to fp8e4 (x 2^20)
     and multiplied against fp8(h) with DoubleRow perf mode -- each matmul
     contracts TWO 128-node k-tiles (probe: 216ns for 2 tiles), so the
     correction costs 16 matmuls/iter.  Effective L precision ~2^-15,
     h ~2^-15: rel err stays ~1e-3 like the baseline.
  3. Decode per gathered chunk is 3 DVE ops (dequant->f32r hi limb, fused
     scalar_tensor_tensor lo limb, int16->fp8 cast) instead of the
     baseline's dequant+copy+sub limb-split; the tail computes
     z^T = W_hh @ y^T with ONE fp32 matmul over the 512-col moving
     (instead of 4 per-j stationary matmuls), then PE-transposes z^T back
     to node-major for tanh/quantize.
  4. int16 wire + split half-AllGathers and the PE warm filler are kept
     from v2.

Measured rel err ~1e-3 (gate 2e-2).
"""

import os
import sys

import numpy as np

sys.path.insert(0, "/opt/trn_rl_repo")

import ml_dtypes

N = 4096
D_IN = 64
H = 64
MAX_ITER = 10
NCORES = 8
ROWS = N // NCORES  # 512
KT = N // 128  # 32 k-tiles over the node dim
KTH = KT // 2  # 16 k-tiles per gather chunk
RT = ROWS // 128  # 4 row-tiles per core chunk
S_LO = float(2**20)  # scale of the accumulated y^T (= S_H * S_L)
S_H = float(2**11)  # carried by the fp16 h limbs (keeps fp16 range safe)
S_L = float(2**9)  # carried by the fp16 L_hi (max |L_hi*S_L| ~ 48 < 65504)
WIRE = 32767.0

_CACHE = {}
LAST_RESULTS = None


def _build_nc():
    import concourse.bacc as bacc
    import concourse.mybir as mybir
    import concourse.tile as tile
    from concourse import masks

    F32 = mybir.dt.float32
    F32R = mybir.dt.float32r
    F16 = mybir.dt.float16
    I16 = mybir.dt.int16
    FP8 = mybir.dt.float8e4
    TANH = mybir.ActivationFunctionType.Tanh
    MULT = mybir.AluOpType.mult
    SUB = mybir.AluOpType.subtract
    ADD = mybir.AluOpType.add
    DR = mybir.MatmulPerfMode.DoubleRow

    nc = bacc.Bacc(None, target_bir_lowering=False, num_devices=NCORES)

    # L^T shard, node axis in GATHER-PERMUTED order (see host code)
    LTR = nc.dram_tensor("LTR", [N, ROWS], F16, kind="ExternalInput")
    LT8 = nc.dram_tensor("LT8", [N, ROWS], FP8, kind="ExternalInput")
    XT = nc.dram_tensor("XT", [D_IN, ROWS], F32, kind="ExternalInput")
    WIH0 = nc.dram_tensor("WIH0T", [D_IN, H], F32, kind="ExternalInput")
    WHH0 = nc.dram_tensor("WHH0T", [H, H], F32, kind="ExternalInput")
    WIH1 = nc.dram_tensor("WIH1T", [H, H], F32, kind="ExternalInput")
    WHH1 = nc.dram_tensor("WHH1T", [H, H], F32, kind="ExternalInput")
    OUT = nc.dram_tensor("OUT", [ROWS, 2 * H], F32, kind="ExternalOutput")

    replica = [list(range(NCORES))]

    with tile.TileContext(nc) as tc:
        with (
            tc.tile_pool(name="cpool", bufs=1) as cpool,
            tc.tile_pool(name="spool", bufs=2) as spool,
            tc.tile_pool(name="ppool", bufs=1, space="PSUM") as ppool,
            tc.tile_pool(name="dpool", bufs=2, space="DRAM") as dpool,
        ):
            # ---- small inputs first (wiu path must not wait on L)
            xt = cpool.tile([D_IN, ROWS], F32)
            nc.sync.dma_start(xt[:], XT.ap())
            wih0 = cpool.tile([D_IN, H], F32)
            nc.sync.dma_start(wih0[:], WIH0.ap())
            whh0 = cpool.tile([H, H], F32)
            nc.sync.dma_start(whh0[:], WHH0.ap())
            wih1 = cpool.tile([H, H], F32)
            nc.sync.dma_start(wih1[:], WIH1.ap())
            whh1 = cpool.tile([H, H], F32)
            nc.sync.dma_start(whh1[:], WHH1.ap())

            # ---- resident L: chunk-split tiles so chunk-A matmuls can
            # start while chunk-B is still loading.
            # ltr[p, k, m] = f32r(L^T[128k+p, m]); k local to the chunk.
            ltr_src = LTR.ap().rearrange("(k p) m -> p k m", p=128)
            lt8_src = LT8.ap().rearrange("(kp i p) m -> p kp i m", p=128, i=2)
            ltr_t = []
            lt8_t = []
            # L load split across both HWDGE engines for max bandwidth; the
            # first collective is gated by this load finishing either way.
            for half in range(2):
                t = cpool.tile(
                    [128, KTH, ROWS], F16, tag=f"ltr{half}", name=f"ltr{half}"
                )
                for q in range(8):
                    ks = slice(2 * q, 2 * (q + 1))
                    gs = slice(KTH * half + 2 * q, KTH * half + 2 * (q + 1))
                    eng = nc.sync if q % 2 == 0 else nc.scalar
                    eng.dma_start(t[:, ks, :], ltr_src[:, gs, :])
                ltr_t.append(t)
                t8 = cpool.tile(
                    [128, KTH // 2, 2, ROWS],
                    FP8,
                    tag=f"lt8{half}",
                    name=f"lt8{half}",
                )
                (nc.scalar if half else nc.sync).dma_start(
                    t8[:],
                    lt8_src[:, (KTH // 2) * half : (KTH // 2) * (half + 1), :, :],
                )
                lt8_t.append(t8)

            # warm the collective path: a tiny AllGather issued first so the
            # one-time ncfw/CC init overlaps the L load instead of stalling
            # iteration 2's first real gather
            # (contents irrelevant -- gathers whatever is in DRAM)
            ccw_in = dpool.tile([1, H], I16, tag="ccw_i", name="ccw_i", bufs=1)
            ccw_out = dpool.tile(
                [NCORES, H], I16, tag="ccw_o", name="ccw_o", bufs=1,
                addr_space="Shared",
            )
            nc.gpsimd.collective_compute(
                "AllGather",
                mybir.AluOpType.bypass,
                replica_groups=replica,
                ins=[ccw_in.opt()],
                outs=[ccw_out.opt()],
            )

            ident_f32 = cpool.tile([128, 128], F32)
            masks.make_identity(nc, ident_f32[:])

            out_stage = cpool.tile([128, RT, 2 * H], F32)

            def warm_pe(n):
                # keep the PE activity window alive across the AllGather
                # hole (idle >3.4us drops the PE clock)
                pwarm = ppool.tile([64, ROWS], F32, tag="warm", name="pwarm", bufs=1)
                for _ in range(n):
                    nc.tensor.matmul(
                        pwarm[0:1, :],
                        ltr_t[0][:, 0, 0:1],
                        ltr_t[0][:, 0, :],
                        start=True,
                        stop=True,
                    )

            def gather(hq, half, tagsuf):
                """AllGather one int16 [128, 2, H] chunk -> DRAM [N/2, H]."""
                cc_in = dpool.tile(
                    [ROWS // 2, H], I16, tag="cc_i" + tagsuf, name="cc_i" + tagsuf
                )
                # SWDGE (gpsimd) queue: never behind the big L loads
                nc.gpsimd.dma_start(
                    cc_in.rearrange("(t p) h -> p t h", p=128),
                    hq[half][:, :, :],
                )
                cc_out = dpool.tile(
                    [N // 2, H],
                    I16,
                    tag="cc_o" + tagsuf,
                    name="cc_o" + tagsuf,
                    addr_space="Shared",
                )
                nc.gpsimd.collective_compute(
                    "AllGather",
                    mybir.AluOpType.bypass,
                    replica_groups=replica,
                    ins=[cc_in.opt()],
                    outs=[cc_out.opt()],
                )
                return cc_out

            BOUNDS = (0, 2, 4, 8, 12, 16)

            def fetch_decode_mm(cc_out, half, tagsuf, p1):
                """DRAM int16 chunk -> decode -> pass1 f32r + pass2 fp8 DR."""
                q16 = spool.tile(
                    [128, KTH, H], I16, tag="q" + tagsuf, name="q" + tagsuf
                )
                src = cc_out.rearrange("(k p) h -> p k h", p=128)
                # split across both engines' dynamic queues
                nc.sync.dma_start(q16[:, 0:1, :], src[:, 0:1, :])
                nc.scalar.dma_start(q16[:, 1:4, :], src[:, 1:4, :])
                nc.sync.dma_start(q16[:, 4:10, :], src[:, 4:10, :])
                nc.scalar.dma_start(q16[:, 10:16, :], src[:, 10:16, :])
                hp = spool.tile(
                    [128, KTH, 2 * H], F16, tag="hp" + tagsuf, name="hp" + tagsuf
                )
                h8 = spool.tile(
                    [128, KTH, H], FP8, tag="h8" + tagsuf, name="h8" + tagsuf
                )
                for c in range(len(BOUNDS) - 1):
                    a, b = BOUNDS[c], BOUNDS[c + 1]
                    ks = slice(a, b)
                    # hi limb: f32r(q / WIRE)
                    nc.vector.tensor_scalar_mul(
                        hp[:, ks, 0:H], q16[:, ks, :], S_H / WIRE
                    )
                    # lo limb: (q / WIRE) - hi
                    nc.vector.scalar_tensor_tensor(
                        hp[:, ks, H : 2 * H],
                        q16[:, ks, :],
                        S_H / WIRE,
                        hp[:, ks, 0:H],
                        MULT,
                        SUB,
                    )
                    # fp8 h for the residual pass
                    nc.vector.tensor_scalar_mul(h8[:, ks, :], q16[:, ks, :], 1.0 / WIRE)
                    # pass 1 (f32r): k-tiles a..b -- keep same-kind matmuls
                    # back-to-back so LDWEIGHTS pipelines.  The accumulation
                    # group is closed by the last DoubleRow matmul below.
                    for k in range(a, b):
                        nc.tensor.matmul(
                            p1[:, :],
                            hp[:, k, :],
                            ltr_t[half][:, k, :],
                            start=(half == 0 and k == 0),
                            stop=False,
                            skip_group_check=True,
                        )
                # pass 2 (fp8 DoubleRow) grouped after pass 1, accumulating
                # straight into p1's hi rows (same scale: L_hi and L_lo both
                # carry S_LO) -- saves one DVE combine op
                for kp in range(KTH // 2):
                    nc.tensor.matmul(
                        p1[0:64, :],
                        h8[:, 2 * kp : 2 * kp + 2, :],
                        lt8_t[half][:, kp, :, :],
                        start=False,
                        stop=(half == 1 and kp == KTH // 2 - 1),
                        perf_mode=DR,
                        skip_group_check=True,
                    )

            def tail(pz, wiuT, quantize):
                """z^T = pz + wiuT -> transpose -> tanh -> (quantize).

                The 1/S_LO rescale of pz is folded into the host-side W_hh
                (whh = W_hh^T / S_LO), so this is a plain add.
                """
                if pz is None:
                    zT = wiuT  # iteration 1: z = wiu
                else:
                    # split the add so j0/j1's transpose can start before the
                    # j2/j3 half is computed
                    zT = spool.tile([64, ROWS], F32, tag="zT", name="zT")
                    nc.vector.tensor_add(
                        zT[:, 0:256], pz[:, 0:256], wiuT[:, 0:256]
                    )
                    nc.vector.tensor_add(
                        zT[:, 256:512], pz[:, 256:512], wiuT[:, 256:512]
                    )
                h_own = spool.tile([128, RT, H], F32, tag="h_own", name="h_own")
                hq = None
                if quantize:
                    # separate A/B wire tiles so the chunk-A gather depends
                    # only on j0/j1
                    hq = [
                        spool.tile([128, 2, H], I16, tag="hqa", name="hqa"),
                        spool.tile([128, 2, H], I16, tag="hqb", name="hqb"),
                    ]
                for j in range(RT):
                    cs = slice(128 * j, 128 * (j + 1))
                    # ptr double-buffered so transpose j+1 does not wait on
                    # tanh j's read
                    ptr = ppool.tile(
                        [128, H], F32, tag=f"ptr{j % 2}", name=f"ptr{j}", bufs=1
                    )
                    nc.tensor.transpose(ptr[:], zT[:, cs], ident_f32[0:64, 0:64])
                    nc.scalar.activation(h_own[:, j, :], ptr[:], TANH)
                    if quantize:
                        nc.vector.tensor_scalar_mul(
                            hq[j // 2][:, j % 2, :], h_own[:, j, :], WIRE
                        )
                return h_own, hq

            def layer(wih, whh, xT_src, out_col, pre_warm):
                # wiu^T = (x @ W_ih^T)^T: one fp32 matmul
                pzw = ppool.tile([64, ROWS], F32, tag="pz", name="pzw", bufs=1)
                nc.tensor.matmul(pzw[:], wih[:], xT_src[:], start=True, stop=True)
                wiuT = spool.tile([64, ROWS], F32, tag="wiuT", name="wiuT")
                nc.vector.tensor_copy(wiuT[:], pzw[:])
                # iteration 1: h = tanh(wiu) -- no recurrent term yet
                h_own, hq = tail(None, wiuT, True)
                warm_pe(pre_warm)

                for _t in range(2, MAX_ITER + 1):
                    p1 = ppool.tile([128, ROWS], F32, tag="p1", name="p1", bufs=1)
                    cc_a = gather(hq, 0, "a")
                    cc_b = gather(hq, 1, "b")
                    warm_pe(10)
                    fetch_decode_mm(cc_a, 0, "a", p1)
                    fetch_decode_mm(cc_b, 1, "b", p1)
                    # combine: y^T*S = p1_hi(+DR) + p1_lo.  The lo-half copy
                    # runs during the DoubleRow pass; only the final add is
                    # exposed.  (Each DVE op may read at most one PSUM input.)
                    t_sb = spool.tile([64, ROWS], F32, tag="t_sb", name="t_sb")
                    nc.vector.tensor_copy(t_sb[:], p1[64:128, :])
                    yT = spool.tile([64, ROWS], F32, tag="yT", name="yT")
                    nc.vector.tensor_add(yT[:], p1[0:64, :], t_sb[:])
                    # z^T (pre-wiu) = W_hh @ y^T: one fp32 matmul
                    pz = ppool.tile([64, ROWS], F32, tag="pz", name="pz", bufs=1)
                    nc.tensor.matmul(pz[:], whh[:], yT[:], start=True, stop=True)
                    h_own, hq = tail(pz, wiuT, _t < MAX_ITER)

                nc.vector.tensor_copy(
                    out_stage[:, :, out_col : out_col + H], h_own[:]
                )
                return h_own

            h1 = layer(wih0, whh0, xt, 0, 4)

            # boundary: h1^T [64, 512] f32 for layer-1's wiu
            ptrb = ppool.tile([64, ROWS], F32, tag="ptrb", name="ptrb", bufs=1)
            for j in range(RT):
                nc.tensor.transpose(
                    ptrb[:, 128 * j : 128 * (j + 1)], h1[:, j, :], ident_f32[:]
                )
            h1T = spool.tile([64, ROWS], F32, tag="h1T", name="h1T")
            nc.vector.tensor_copy(h1T[:], ptrb[:])

            layer(wih1, whh1, h1T, H, 4)

            out_dst = OUT.ap().rearrange("(t p) h -> p t h", p=128)
            nc.sync.dma_start(out_dst[:, 0:2, :], out_stage[:, 0:2, :])
            nc.scalar.dma_start(out_dst[:, 2:4, :], out_stage[:, 2:4, :])

    nc.compile()
    return nc


def _get_nc():
    if "nc" not in _CACHE:
        _CACHE["nc"] = _build_nc()
    return _CACHE["nc"]


def _ensure_ntff_hook():
    """bass_utils needs antenv.axon_hooks for trace=True under axon; the
    agent image's antenv lacks it.  Register an equivalent shim in
    sys.modules backed by ctypes calls into libaxon_pjrt.so."""
    import types

    try:
        import antenv.axon_hooks  # noqa: F401

        return
    except ImportError:
        pass
    mod = types.ModuleType("antenv.axon_hooks")
    state = {"hook": None, "tried": False}

    def set_axon_ntff_profile_hook(hook):
        state["hook"] = hook

    def get_axon_ntff_profile_hook():
        if state["hook"] is None and not state["tried"]:
            state["tried"] = True
            try:
                from trn_agent_boot.trn_boot import _ntff_profile_via_ctypes

                state["hook"] = _ntff_profile_via_ctypes(
                    "/opt/axon/libaxon_pjrt.so"
                )
            except Exception:
                state["hook"] = None
        return state["hook"]

    mod.set_axon_ntff_profile_hook = set_axon_ntff_profile_hook
    mod.get_axon_ntff_profile_hook = get_axon_ntff_profile_hook
    sys.modules["antenv.axon_hooks"] = mod


# gather-order permutation of the node axis: chunk A = every rank's rows
# 0:256, chunk B = every rank's rows 256:512
_PERM = np.concatenate(
    [np.arange(512 * r, 512 * r + 256) for r in range(NCORES)]
    + [np.arange(512 * r + 256, 512 * (r + 1)) for r in range(NCORES)]
)


def _tf32_round(x):
    """Round f32 to 10 explicit mantissa bits (round-to-nearest-even)."""
    b = np.ascontiguousarray(x, dtype=np.float32).view(np.uint32)
    keep = 13
    rnd = ((b >> keep) & 1).astype(np.uint32) + np.uint32((1 << (keep - 1)) - 1)
    return ((b + rnd) & ~np.uint32((1 << keep) - 1)).view(np.float32)


def kernel(X, L, W_ih0, W_hh0, W_ih1, W_hh1):
    global LAST_RESULTS
    _ensure_ntff_hook()
    from concourse.bass_utils import run_bass_kernel_spmd

    nc = _get_nc()
    f32 = np.float32
    fp8 = ml_dtypes.float8_e4m3

    wih0 = np.ascontiguousarray(np.asarray(W_ih0).T).astype(f32)
    wih1 = np.ascontiguousarray(np.asarray(W_ih1).T).astype(f32)
    # y^T arrives scaled by S_LO (L pre-scaled on host); fold the 1/S_LO
    # rescale into the recurrent weights
    whh0 = np.ascontiguousarray(np.asarray(W_hh0).T).astype(f32) / np.float32(S_LO)
    whh1 = np.ascontiguousarray(np.asarray(W_hh1).T).astype(f32) / np.float32(S_LO)

    Lf = np.asarray(L, dtype=f32)
    in_maps = []
    for c in range(NCORES):
        rows = slice(ROWS * c, ROWS * (c + 1))
        ltc = np.ascontiguousarray(Lf[rows, :].T[_PERM, :])  # [N, ROWS] f32
        # L_hi: tf32-rounded then stored as fp16 * S_L (exact for 10-bit
        # mantissas except fp16 subnormals, which the residual absorbs)
        lhi16 = (_tf32_round(ltc) * np.float32(S_L)).astype(np.float16)
        llo = (ltc - lhi16.astype(f32) / np.float32(S_L)) * S_LO
        in_maps.append(
            {
                "LTR": lhi16,
                "LT8": llo.astype(fp8),
                "XT": np.ascontiguousarray(np.asarray(X)[rows, :].T).astype(f32),
                "WIH0T": wih0,
                "WHH0T": whh0,
                "WIH1T": wih1,
                "WHH1T": whh1,
            }
        )

    trace = bool(int(os.environ.get("KERNEL_TRACE", "0")))
    res = run_bass_kernel_spmd(
        nc, in_maps, core_ids=list(range(NCORES)), trace=trace
    )
    LAST_RESULTS = res
    out = np.concatenate([r["OUT"] for r in res.results], axis=0)
    return np.asarray(out, dtype=np.float32)


# revision 45
# speedup vs baseline: 1.0115x; 1.0115x over previous
"""DeepGESNCell kernel v3 for 8 TRN2 NeuronCores.

h <- tanh(wiu + L @ (h @ W_hh^T)) iterated 10x, two layers, out = [h1|h2].

v3 vs the 794us v2 baseline (2-pass packed double-bf16):
  1. Pass 1 of the big matmul runs in float32r (TF32-like, 1 cycle/row for
     512-col moving -- probe-measured 228ns/matmul, same as bf16): the
     stationary packs [h_hi | h_lo] f32r limbs (h exact to wire precision),
     the moving is L_hi = tf32-rounded L (10 explicit mantissa bits,
     pre-rounded on host so the HW f32r rounding is a no-op).
     32 matmuls/iter instead of 64.
  2. Pass 2 adds the L residual: L_lo = L - L_hi quantized to fp8e4 (x 2^20)
     and multiplied against fp8(h) with DoubleRow perf mode -- each matmul
     contracts TWO 128-node k-tiles (probe: 216ns for 2 tiles), so the
     correction costs 16 matmuls/iter.  Effective L precision ~2^-15,
     h ~2^-15: rel err stays ~1e-3 like the baseline.
  3. Decode per gathered chunk is 3 DVE ops (dequant->f32r hi limb, fused
     scalar_tensor_tensor lo limb, int16->fp8 cast) instead of the
     baseline's dequant+copy+sub limb-split; the tail computes
     z^T = W_hh @ y^T with ONE fp32 matmul over the 512-col moving
     (instead of 4 per-j stationary matmuls), then PE-transposes z^T back
     to node-major for tanh/quantize.
  4. int16 wire + split half-AllGathers and the PE warm filler are kept
     from v2.

Measured rel err ~1e-3 (gate 2e-2).
"""

import os
import sys

import numpy as np

sys.path.insert(0, "/opt/trn_rl_repo")

import ml_dtypes

N = 4096
D_IN = 64
H = 64
MAX_ITER = 10
NCORES = 8
ROWS = N // NCORES  # 512
KT = N // 128  # 32 k-tiles over the node dim
KTH = KT // 2  # 16 k-tiles per gather chunk
RT = ROWS // 128  # 4 row-tiles per core chunk
S_LO = float(2**20)  # scale of the accumulated y^T (= S_H * S_L)
S_H = float(2**11)  # carried by the fp16 h limbs (keeps fp16 range safe)
S_L = float(2**9)  # carried by the fp16 L_hi (max |L_hi*S_L| ~ 48 < 65504)
WIRE = 32767.0

_CACHE = {}
LAST_RESULTS = None


def _build_nc():
    import concourse.bacc as bacc
    import concourse.mybir as mybir
    import concourse.tile as tile
    from concourse import masks

    F32 = mybir.dt.float32
    F32R = mybir.dt.float32r
    F16 = mybir.dt.float16
    I16 = mybir.dt.int16
    FP8 = mybir.dt.float8e4
    TANH = mybir.ActivationFunctionType.Tanh
    MULT = mybir.AluOpType.mult
    SUB = mybir.AluOpType.subtract
    ADD = mybir.AluOpType.add
    DR = mybir.MatmulPerfMode.DoubleRow

    nc = bacc.Bacc(None, target_bir_lowering=False, num_devices=NCORES)

    # L^T shard, node axis in GATHER-PERMUTED order (see host code)
    LTR = nc.dram_tensor("LTR", [N, ROWS], F32R, kind="ExternalInput")
    LT8 = nc.dram_tensor("LT8", [N, ROWS], FP8, kind="ExternalInput")
    XT = nc.dram_tensor("XT", [D_IN, ROWS], F32, kind="ExternalInput")
    WIH0 = nc.dram_tensor("WIH0T", [D_IN, H], F32, kind="ExternalInput")
    WHH0 = nc.dram_tensor("WHH0T", [H, H], F32, kind="ExternalInput")
    WIH1 = nc.dram_tensor("WIH1T", [H, H], F32, kind="ExternalInput")
    WHH1 = nc.dram_tensor("WHH1T", [H, H], F32, kind="ExternalInput")
    OUT = nc.dram_tensor("OUT", [ROWS, 2 * H], F32, kind="ExternalOutput")

    replica = [list(range(NCORES))]

    with tile.TileContext(nc) as tc:
        with (
            tc.tile_pool(name="cpool", bufs=1) as cpool,
            tc.tile_pool(name="spool", bufs=2) as spool,
            tc.tile_pool(name="ppool", bufs=1, space="PSUM") as ppool,
            tc.tile_pool(name="dpool", bufs=2, space="DRAM") as dpool,
        ):
            # ---- small inputs first (wiu path must not wait on L)
            xt = cpool.tile([D_IN, ROWS], F32)
            nc.sync.dma_start(xt[:], XT.ap())
            wih0 = cpool.tile([D_IN, H], F32)
            nc.sync.dma_start(wih0[:], WIH0.ap())
            whh0 = cpool.tile([H, H], F32)
            nc.sync.dma_start(whh0[:], WHH0.ap())
            wih1 = cpool.tile([H, H], F32)
            nc.sync.dma_start(wih1[:], WIH1.ap())
            whh1 = cpool.tile([H, H], F32)
            nc.sync.dma_start(whh1[:], WHH1.ap())

            # ---- resident L: chunk-split tiles so chunk-A matmuls can
            # start while chunk-B is still loading.
            # ltr[p, k, m] = f32r(L^T[128k+p, m]); k local to the chunk.
            ltr_src = LTR.ap().rearrange("(k p) m -> p k m", p=128)
            lt8_src = LT8.ap().rearrange("(kp i p) m -> p kp i m", p=128, i=2)
            ltr_t = []
            lt8_t = []
            # L load split across both HWDGE engines for max bandwidth; the
            # first collective is gated by this load finishing either way.
            for half in range(2):
                t = cpool.tile(
                    [128, KTH, ROWS], F32R, tag=f"ltr{half}", name=f"ltr{half}"
                )
                for q in range(8):
                    ks = slice(2 * q, 2 * (q + 1))
                    gs = slice(KTH * half + 2 * q, KTH * half + 2 * (q + 1))
                    eng = nc.sync if q % 2 == 0 else nc.scalar
                    eng.dma_start(t[:, ks, :], ltr_src[:, gs, :])
                ltr_t.append(t)
                t8 = cpool.tile(
                    [128, KTH // 2, 2, ROWS],
                    FP8,
                    tag=f"lt8{half}",
                    name=f"lt8{half}",
                )
                (nc.scalar if half else nc.sync).dma_start(
                    t8[:],
                    lt8_src[:, (KTH // 2) * half : (KTH // 2) * (half + 1), :, :],
                )
                lt8_t.append(t8)

            ident_f32 = cpool.tile([128, 128], F32)
            masks.make_identity(nc, ident_f32[:])

            out_stage = cpool.tile([128, RT, 2 * H], F32)

            def warm_pe(n):
                # keep the PE activity window alive across the AllGather
                # hole (idle >3.4us drops the PE clock)
                pwarm = ppool.tile([64, ROWS], F32, tag="warm", name="pwarm", bufs=1)
                for _ in range(n):
                    nc.tensor.matmul(
                        pwarm[0:1, :],
                        ltr_t[0][:, 0, 0:1],
                        ltr_t[0][:, 0, :],
                        start=True,
                        stop=True,
                    )

            def gather(hq, half, tagsuf):
                """AllGather one int16 [128, 2, H] chunk -> DRAM [N/2, H]."""
                cc_in = dpool.tile(
                    [ROWS // 2, H], I16, tag="cc_i" + tagsuf, name="cc_i" + tagsuf
                )
                # SWDGE (gpsimd) queue: never behind the big L loads
                nc.gpsimd.dma_start(
                    cc_in.rearrange("(t p) h -> p t h", p=128),
                    hq[half][:, :, :],
                )
                cc_out = dpool.tile(
                    [N // 2, H],
                    I16,
                    tag="cc_o" + tagsuf,
                    name="cc_o" + tagsuf,
                    addr_space="Shared",
                )
                nc.gpsimd.collective_compute(
                    "AllGather",
                    mybir.AluOpType.bypass,
                    replica_groups=replica,
                    ins=[cc_in.opt()],
                    outs=[cc_out.opt()],
                )
                return cc_out

            BOUNDS = (0, 2, 4, 8, 12, 16)

            def fetch_decode_mm(cc_out, half, tagsuf, p1):
                """DRAM int16 chunk -> decode -> pass1 f32r + pass2 fp8 DR."""
                q16 = spool.tile(
                    [128, KTH, H], I16, tag="q" + tagsuf, name="q" + tagsuf
                )
                src = cc_out.rearrange("(k p) h -> p k h", p=128)
                # split across both engines' dynamic queues
                nc.sync.dma_start(q16[:, 0:1, :], src[:, 0:1, :])
                nc.scalar.dma_start(q16[:, 1:4, :], src[:, 1:4, :])
                nc.sync.dma_start(q16[:, 4:10, :], src[:, 4:10, :])
                nc.scalar.dma_start(q16[:, 10:16, :], src[:, 10:16, :])
                hp = spool.tile(
                    [128, KTH, 2 * H], F32R, tag="hp" + tagsuf, name="hp" + tagsuf
                )
                h8 = spool.tile(
                    [128, KTH, H], FP8, tag="h8" + tagsuf, name="h8" + tagsuf
                )
                for c in range(len(BOUNDS) - 1):
                    a, b = BOUNDS[c], BOUNDS[c + 1]
                    ks = slice(a, b)
                    # hi limb: f32r(q / WIRE)
                    nc.vector.tensor_scalar_mul(
                        hp[:, ks, 0:H], q16[:, ks, :], 1.0 / WIRE
                    )
                    # lo limb: (q / WIRE) - hi
                    nc.vector.scalar_tensor_tensor(
                        hp[:, ks, H : 2 * H],
                        q16[:, ks, :],
                        1.0 / WIRE,
                        hp[:, ks, 0:H],
                        MULT,
                        SUB,
                    )
                    # fp8 h for the residual pass
                    nc.vector.tensor_scalar_mul(h8[:, ks, :], q16[:, ks, :], 1.0 / WIRE)
                    # pass 1 (f32r): k-tiles a..b -- keep same-kind matmuls
                    # back-to-back so LDWEIGHTS pipelines.  The accumulation
                    # group is closed by the last DoubleRow matmul below.
                    for k in range(a, b):
                        nc.tensor.matmul(
                            p1[:, :],
                            hp[:, k, :],
                            ltr_t[half][:, k, :],
                            start=(half == 0 and k == 0),
                            stop=False,
                            skip_group_check=True,
                        )
                # pass 2 (fp8 DoubleRow) grouped after pass 1, accumulating
                # straight into p1's hi rows (same scale: L_hi and L_lo both
                # carry S_LO) -- saves one DVE combine op
                for kp in range(KTH // 2):
                    nc.tensor.matmul(
                        p1[0:64, :],
                        h8[:, 2 * kp : 2 * kp + 2, :],
                        lt8_t[half][:, kp, :, :],
                        start=False,
                        stop=(half == 1 and kp == KTH // 2 - 1),
                        perf_mode=DR,
                        skip_group_check=True,
                    )

            def tail(pz, wiuT, quantize):
                """z^T = pz + wiuT -> transpose -> tanh -> (quantize).

                The 1/S_LO rescale of pz is folded into the host-side W_hh
                (whh = W_hh^T / S_LO), so this is a plain add.
                """
                if pz is None:
                    zT = wiuT  # iteration 1: z = wiu
                else:
                    # split the add so j0/j1's transpose can start before the
                    # j2/j3 half is computed
                    zT = spool.tile([64, ROWS], F32, tag="zT", name="zT")
                    nc.vector.tensor_add(
                        zT[:, 0:256], pz[:, 0:256], wiuT[:, 0:256]
                    )
                    nc.vector.tensor_add(
                        zT[:, 256:512], pz[:, 256:512], wiuT[:, 256:512]
                    )
                h_own = spool.tile([128, RT, H], F32, tag="h_own", name="h_own")
                hq = None
                if quantize:
                    # separate A/B wire tiles so the chunk-A gather depends
                    # only on j0/j1
                    hq = [
                        spool.tile([128, 2, H], I16, tag="hqa", name="hqa"),
                        spool.tile([128, 2, H], I16, tag="hqb", name="hqb"),
                    ]
                for j in range(RT):
                    cs = slice(128 * j, 128 * (j + 1))
                    # ptr double-buffered so transpose j+1 does not wait on
                    # tanh j's read
                    ptr = ppool.tile(
                        [128, H], F32, tag=f"ptr{j % 2}", name=f"ptr{j}", bufs=1
                    )
                    nc.tensor.transpose(ptr[:], zT[:, cs], ident_f32[0:64, 0:64])
                    nc.scalar.activation(h_own[:, j, :], ptr[:], TANH)
                    if quantize:
                        nc.vector.tensor_scalar_mul(
                            hq[j // 2][:, j % 2, :], h_own[:, j, :], WIRE
                        )
                return h_own, hq

            def layer(wih, whh, xT_src, out_col, pre_warm):
                # wiu^T = (x @ W_ih^T)^T: one fp32 matmul
                pzw = ppool.tile([64, ROWS], F32, tag="pz", name="pzw", bufs=1)
                nc.tensor.matmul(pzw[:], wih[:], xT_src[:], start=True, stop=True)
                wiuT = spool.tile([64, ROWS], F32, tag="wiuT", name="wiuT")
                nc.vector.tensor_copy(wiuT[:], pzw[:])
                # iteration 1: h = tanh(wiu) -- no recurrent term yet
                h_own, hq = tail(None, wiuT, True)
                warm_pe(pre_warm)

                for _t in range(2, MAX_ITER + 1):
                    p1 = ppool.tile([128, ROWS], F32, tag="p1", name="p1", bufs=1)
                    cc_a = gather(hq, 0, "a")
                    cc_b = gather(hq, 1, "b")
                    warm_pe(10)
                    fetch_decode_mm(cc_a, 0, "a", p1)
                    fetch_decode_mm(cc_b, 1, "b", p1)
                    # combine: y^T*S = p1_hi(+DR) + p1_lo.  The lo-half copy
                    # runs during the DoubleRow pass; only the final add is
                    # exposed.  (Each DVE op may read at most one PSUM input.)
                    t_sb = spool.tile([64, ROWS], F32, tag="t_sb", name="t_sb")
                    nc.vector.tensor_copy(t_sb[:], p1[64:128, :])
                    yT = spool.tile([64, ROWS], F32, tag="yT", name="yT")
                    nc.vector.tensor_add(yT[:], p1[0:64, :], t_sb[:])
                    # z^T (pre-wiu) = W_hh @ y^T: one fp32 matmul
                    pz = ppool.tile([64, ROWS], F32, tag="pz", name="pz", bufs=1)
                    nc.tensor.matmul(pz[:], whh[:], yT[:], start=True, stop=True)
                    h_own, hq = tail(pz, wiuT, _t < MAX_ITER)

                nc.vector.tensor_copy(
                    out_stage[:, :, out_col : out_col + H], h_own[:]
                )
                return h_own

            h1 = layer(wih0, whh0, xt, 0, 4)

            # boundary: h1^T [64, 512] f32 for layer-1's wiu
            ptrb = ppool.tile([64, ROWS], F32, tag="ptrb", name="ptrb", bufs=1)
            for j in range(RT):
                nc.tensor.transpose(
                    ptrb[:, 128 * j : 128 * (j + 1)], h1[:, j, :], ident_f32[:]
                )
            h1T = spool.tile([64, ROWS], F32, tag="h1T", name="h1T")
            nc.vector.tensor_copy(h1T[:], ptrb[:])

            layer(wih1, whh1, h1T, H, 4)

            out_dst = OUT.ap().rearrange("(t p) h -> p t h", p=128)
            nc.sync.dma_start(out_dst[:, 0:2, :], out_stage[:, 0:2, :])
            nc.scalar.dma_start(out_dst[:, 2:4, :], out_stage[:, 2:4, :])

    nc.compile()
    return nc


def _get_nc():
    if "nc" not in _CACHE:
        _CACHE["nc"] = _build_nc()
    return _CACHE["nc"]


def _ensure_ntff_hook():
    """bass_utils needs antenv.axon_hooks for trace=True under axon; the
    agent image's antenv lacks it.  Register an equivalent shim in
    sys.modules backed by ctypes calls into libaxon_pjrt.so."""
    import types

    try:
        import antenv.axon_hooks  # noqa: F401

        return
    except ImportError:
        pass
    mod = types.ModuleType("antenv.axon_hooks")
    state = {"hook": None, "tried": False}

    def set_axon_ntff_profile_hook(hook):
        state["hook"] = hook

    def get_axon_ntff_profile_hook():
        if state["hook"] is None and not state["tried"]:
            state["tried"] = True
            try:
                from trn_agent_boot.trn_boot import _ntff_profile_via_ctypes

                state["hook"] = _ntff_profile_via_ctypes(
                    "/opt/axon/libaxon_pjrt.so"
                )
            except Exception:
                state["hook"] = None
        return state["hook"]

    mod.set_axon_ntff_profile_hook = set_axon_ntff_profile_hook
    mod.get_axon_ntff_profile_hook = get_axon_ntff_profile_hook
    sys.modules["antenv.axon_hooks"] = mod


# gather-order permutation of the node axis: chunk A = every rank's rows
# 0:256, chunk B = every rank's rows 256:512
_PERM = np.concatenate(
    [np.arange(512 * r, 512 * r + 256) for r in range(NCORES)]
    + [np.arange(512 * r + 256, 512 * (r + 1)) for r in range(NCORES)]
)


def _tf32_round(x):
    """Round f32 to 10 explicit mantissa bits (round-to-nearest-even)."""
    b = np.ascontiguousarray(x, dtype=np.float32).view(np.uint32)
    keep = 13
    rnd = ((b >> keep) & 1).astype(np.uint32) + np.uint32((1 << (keep - 1)) - 1)
    return ((b + rnd) & ~np.uint32((1 << keep) - 1)).view(np.float32)


def kernel(X, L, W_ih0, W_hh0, W_ih1, W_hh1):
    global LAST_RESULTS
    _ensure_ntff_hook()
    from concourse.bass_utils import run_bass_kernel_spmd

    nc = _get_nc()
    f32 = np.float32
    fp8 = ml_dtypes.float8_e4m3

    wih0 = np.ascontiguousarray(np.asarray(W_ih0).T).astype(f32)
    wih1 = np.ascontiguousarray(np.asarray(W_ih1).T).astype(f32)
    # y^T arrives scaled by S_LO (L pre-scaled on host); fold the 1/S_LO
    # rescale into the recurrent weights
    whh0 = np.ascontiguousarray(np.asarray(W_hh0).T).astype(f32) / np.float32(S_LO)
    whh1 = np.ascontiguousarray(np.asarray(W_hh1).T).astype(f32) / np.float32(S_LO)

    Lf = np.asarray(L, dtype=f32)
    in_maps = []
    for c in range(NCORES):
        rows = slice(ROWS * c, ROWS * (c + 1))
        ltc = np.ascontiguousarray(Lf[rows, :].T[_PERM, :])  # [N, ROWS] f32
        lhi = _tf32_round(ltc)
        llo = (ltc - lhi) * S_LO
        in_maps.append(
            {
                "LTR": lhi * np.float32(S_LO),
                "LT8": llo.astype(fp8),
                "XT": np.ascontiguousarray(np.asarray(X)[rows, :].T).astype(f32),
                "WIH0T": wih0,
                "WHH0T": whh0,
                "WIH1T": wih1,
                "WHH1T": whh1,
            }
        )

    trace = bool(int(os.environ.get("KERNEL_TRACE", "0")))
    res = run_bass_kernel_spmd(
        nc, in_maps, core_ids=list(range(NCORES)), trace=trace
    )
    LAST_RESULTS = res
    out = np.concatenate([r["OUT"] for r in res.results], axis=0)
    return np.asarray(out, dtype=np.float32)
